# revision 4
# baseline (speedup 1.0000x reference)
"""Trainium2 Bass kernel for nn_ByteBitwiseFFN.

Reference semantics (per token, D=128 features):
  a = argmax(x[4:20]) + 16*argmax(x[20:36])
  b = argmax(x[36:52]) + 16*argmax(x[52:68])
  res = AND/OR/XOR LUT[a,b] picked by flags x[1]>0.5 / x[2]>0.5 / x[3]>0.5
        (priority AND, OR, XOR; XOR value also used when no flag set)
  active = (x[0]>=0.5) & any-flag; w = active ? 2 : 0
  out = x; out[68 + (res&15)] += w; out[84 + (res>>4)] += w

Key identities:
* Bitwise ops factor over nibbles, so the 256x256 LUTs are never needed:
  res&15 = op(a_lo, b_lo), res>>4 = op(a_hi, b_hi), and for 4-bit operands
  op(u, v) = alpha*(u+v) + beta*(u AND v) with (alpha, beta) =
  (0,1) AND / (1,-1) OR / (1,-2) XOR.  The AND is one int16 bitwise_and.
* First-occurrence argmax without any compare instruction:
  min over j of ((max - x_j) + (j-16)*2^-100) is (argmax-16)*2^-100 --
  the subtract term is 0 exactly at max positions and otherwise dwarfs
  the index encoding (data value gaps are > 1e-6 >> 16*2^-100).

Sharding: pure data parallel over tokens; each of the 8 cores gets
131072/8 = 16384 tokens as its own ExternalInput.

Engine split (from per-op HW microbenchmarks):
  DVE    - the two 16-wide reductions (no 2x mode exists: 2.27us each),
           compares (is_ge / is_equal), int16 bitwise_and, casts, and the
           small per-token algebra (batched across chunk pairs to halve
           instruction-overhead).
  GpSimd - big streaming passes where its f32->bf16 path is fast:
           d = max-x (1.6us), cand = d+idx (3.6us), and the two f32
           one-hot accumulates (1.15us each).
  ACT    - affine rescales via ACTIVATE Copy (scale*x+bias, ~0.4us).
  DMA    - contiguous 2MB chunk loads/stores via sync queue.
"""

import sys

if "/opt/trn_rl_repo" not in sys.path:
    sys.path.insert(0, "/opt/trn_rl_repo")

import numpy as np

B, S, D = 16, 8192, 128
N_CORES = 8
TOK = B * S                      # 131072 tokens
TOK_PER_CORE = TOK // N_CORES    # 16384
P = 128                          # SBUF partitions

OUT_LO, OUT_HI = 68, 84
EPS = 2.0 ** -100                # index encoding scale for the argmax trick


def build_program(tok_per_core=TOK_PER_CORE, t_per_chunk=32, group=2):
    """Build + compile the single-core SPMD Bass program.

    Layout: the core's [tok_per_core, 128] slab is processed in chunks of
    128*T tokens (contiguous DRAM block <-> SBUF tile [128, T*128];
    partition p holds T consecutive tokens).  Chunks are grouped in pairs:
    the heavy streaming passes run per chunk, the small per-token algebra
    runs once per group on the concatenated results.
    """
    import concourse.bass as bass  # noqa: F401
    from concourse import bacc, mybir, tile

    f32 = mybir.dt.float32
    bf16 = mybir.dt.bfloat16
    i16 = mybir.dt.int16
    i32 = mybir.dt.int32
    Op = mybir.AluOpType
    AF = mybir.ActivationFunctionType
    X = mybir.AxisListType.X

    T = t_per_chunk
    chunk_tok = P * T
    assert tok_per_core % (chunk_tok * group) == 0
    n_groups = tok_per_core // (chunk_tok * group)
    GT = group * T                     # tokens-per-partition in one group

    nc = bacc.Bacc(
        "TRN2",
        target_bir_lowering=False,
        debug=False,
        enable_asserts=True,
        num_devices=N_CORES,
    )
    x_dram = nc.dram_tensor("x", [tok_per_core, D], f32, kind="ExternalInput").ap()
    y_dram = nc.dram_tensor("y", [tok_per_core, D], f32, kind="ExternalOutput").ap()

    with tile.TileContext(nc) as tc:
        with (
            tc.tile_pool(name="consts", bufs=1) as cpool,
            tc.tile_pool(name="xtiles", bufs=3) as xpool,
            tc.tile_pool(name="big", bufs=2) as bp,
            tc.tile_pool(name="small", bufs=2) as sp,
        ):
            v = nc.vector
            g = nc.gpsimd
            a = nc.scalar

            # --- constants -------------------------------------------------
            idx_i = cpool.tile([P, 16], i32)
            nc.gpsimd.iota(idx_i[:], [[1, 16]], base=0, channel_multiplier=0)
            idx16 = cpool.tile([P, 16], bf16)
            v.tensor_copy(idx16[:], idx_i[:])
            # (idx - 16) * 2^-100, exact in bf16
            idxe = cpool.tile([P, 16], bf16)
            v.tensor_scalar(idxe[:], idx16[:], -16.0, EPS, Op.add, Op.mult)

            idxe_b = idxe.unsqueeze(1).unsqueeze(1).broadcast_to([P, T, 4, 16])
            idx16_gb = idx16.unsqueeze(1).broadcast_to([P, GT, 16])

            for gi in range(n_groups):
                xts = []
                # group-wide result tiles, written per chunk, read batched
                am_all = sp.tile([P, GT * 4], bf16, name="am_all")
                am_g = am_all.rearrange("p (t g) -> p t g", g=4)
                fl_all = sp.tile([P, GT * 4], bf16, name="fl_all")
                fl_g = fl_all.rearrange("p (t f) -> p t f", f=4)

                for ci in range(group):
                    i = gi * group + ci
                    xt = xpool.tile([P, T * D], f32, name="xt")
                    xts.append(xt)
                    src = x_dram[i * chunk_tok : (i + 1) * chunk_tok, :].rearrange(
                        "(p t) f -> p (t f)", p=P
                    )
                    nc.sync.dma_start(xt[:], src)

                    x3 = xt.rearrange("p (t f) -> p t f", f=D)
                    nib = x3[:, :, 4:68].rearrange("p t (g n) -> p t g n", n=16)

                    # --- first-occurrence argmax of each 16-wide field -----
                    rmax = bp.tile([P, T * 4], f32, name="rmax")
                    rmax3 = rmax.rearrange("p (t g) -> p t g", g=4)
                    v.tensor_reduce(rmax3, nib, axis=X, op=Op.max)

                    dsub = bp.tile([P, T * 64], bf16, name="dsub")
                    dsub4 = dsub.rearrange("p (t g n) -> p t g n", g=4, n=16)
                    g.tensor_tensor(
                        dsub4,
                        rmax3.unsqueeze(3).broadcast_to([P, T, 4, 16]),
                        nib,
                        Op.subtract,
                    )

                    cand = bp.tile([P, T * 64], bf16, name="cand")
                    cand4 = cand.rearrange("p (t g n) -> p t g n", g=4, n=16)
                    g.tensor_tensor(cand4, dsub4, idxe_b, Op.add)

                    # am = (argmax - 16) * 2^-100, into the group tile
                    v.tensor_reduce(
                        am_g[:, ci * T : (ci + 1) * T, :], cand4, axis=X, op=Op.min
                    )
                    # flags (>=0.5) for cols 0..3, into the group tile
                    v.tensor_scalar(
                        fl_g[:, ci * T : (ci + 1) * T, :],
                        x3[:, :, 0:4],
                        0.5,
                        None,
                        Op.is_ge,
                    )

                # --- batched per-token algebra over the whole group --------
                nv = sp.tile([P, GT * 4], bf16, name="nv")
                nv3 = nv.rearrange("p (t g) -> p t g", g=4)
                a.activation(nv3, am_g, AF.Copy, bias=16.0, scale=2.0 ** 100)
                nvi = sp.tile([P, GT * 4], i16, name="nvi")
                nvi3 = nvi.rearrange("p (t g) -> p t g", g=4)
                v.tensor_copy(nvi3, nv3)

                qi = sp.tile([P, GT * 2], i16, name="qi")
                qi3 = qi.rearrange("p (t f) -> p t f", f=2)
                v.tensor_tensor(qi3, nvi3[:, :, 0:2], nvi3[:, :, 2:4], Op.bitwise_and)
                qq = sp.tile([P, GT * 2], bf16, name="qq")
                qq3 = qq.rearrange("p (t f) -> p t f", f=2)
                v.tensor_copy(qq3, qi3)

                ss = sp.tile([P, GT * 2], bf16, name="ss")
                ss3 = ss.rearrange("p (t f) -> p t f", f=2)
                v.tensor_tensor(ss3, nv3[:, :, 0:2], nv3[:, :, 2:4], Op.add)

                mk = fl_g[:, :, 0:1]
                ia = fl_g[:, :, 1:2]
                io = fl_g[:, :, 2:3]
                ix = fl_g[:, :, 3:4]

                def tmp1(nm):
                    t_ = sp.tile([P, GT], bf16, name=nm)
                    return t_.unsqueeze(2)  # [P, GT, 1]

                alpha = tmp1("alpha")     # 1 - is_and
                a.activation(alpha, ia, AF.Copy, bias=1.0, scale=-1.0)
                s1 = tmp1("s1")           # 3 - is_or
                a.activation(s1, io, AF.Copy, bias=3.0, scale=-1.0)
                s3 = tmp1("s3")           # is_or - 2
                a.activation(s3, io, AF.Copy, bias=-2.0, scale=1.0)
                s2 = tmp1("s2")
                v.tensor_tensor(s2, ia, s1, Op.mult)
                beta = tmp1("beta")       # 1 / -1 / -2
                v.tensor_tensor(beta, s2, s3, Op.add)
                f1 = tmp1("f1")
                v.tensor_tensor(f1, ia, io, Op.add)
                f2 = tmp1("f2")
                v.tensor_tensor(f2, f1, ix, Op.add)
                f3 = tmp1("f3")           # any flag = min(1, ia+io+ix)
                v.tensor_scalar(f3, f2, 1.0, None, Op.min)
                act = tmp1("act")
                v.tensor_tensor(act, mk, f3, Op.mult)

                def tmp2(nm):
                    t_ = sp.tile([P, GT * 2], bf16, name=nm)
                    return t_.rearrange("p (t f) -> p t f", f=2)

                c1 = tmp2("c1")
                v.tensor_tensor(c1, ss3, alpha.broadcast_to([P, GT, 2]), Op.mult)
                c2 = tmp2("c2")
                v.tensor_tensor(c2, qq3, beta.broadcast_to([P, GT, 2]), Op.mult)
                # res + 16*(1-active): out of 0..15 when inactive
                c3 = tmp2("c3")
                v.scalar_tensor_tensor(
                    c3, act.broadcast_to([P, GT, 2]), -16.0, c1, Op.mult, Op.add
                )
                resg = tmp2("resg")
                v.scalar_tensor_tensor(resg, c2, 16.0, c3, Op.add, Op.add)

                # --- one-hot (batched): eq, then 2x scale on ACT -----------
                e2s = []
                for h in range(2):
                    eqh = sp.tile([P, GT * 16], bf16, name=f"eqh{h}")
                    eqh3 = eqh.rearrange("p (t n) -> p t n", n=16)
                    v.tensor_tensor(
                        eqh3,
                        idx16_gb,
                        resg[:, :, h : h + 1].broadcast_to([P, GT, 16]),
                        Op.is_equal,
                    )
                    e2 = sp.tile([P, GT * 16], bf16, name=f"e2{h}")
                    e23 = e2.rearrange("p (t n) -> p t n", n=16)
                    a.activation(e23, eqh3, AF.Copy, bias=0.0, scale=2.0)
                    e2s.append(e23)

                # --- accumulate into x and store, per chunk ----------------
                for ci in range(group):
                    i = gi * group + ci
                    x3 = xts[ci].rearrange("p (t f) -> p t f", f=D)
                    for h, off in enumerate((OUT_LO, OUT_HI)):
                        xs = x3[:, :, off : off + 16]
                        g.tensor_tensor(
                            xs, xs, e2s[h][:, ci * T : (ci + 1) * T, :], Op.add
                        )
                    dst = y_dram[i * chunk_tok : (i + 1) * chunk_tok, :].rearrange(
                        "(p t) f -> p (t f)", p=P
                    )
                    nc.sync.dma_start(dst, xts[ci][:])

    nc.compile()
    return nc


_compiled = None


def _get_compiled():
    global _compiled
    if _compiled is None:
        _compiled = build_program()
    return _compiled


def run_on_hw(nc, shards, trace=False, **kw):
    from concourse.bass_utils import run_bass_kernel_spmd

    return run_bass_kernel_spmd(
        nc, [{"x": s} for s in shards], list(range(N_CORES)), trace=trace, **kw
    )


def kernel(x_bd, and_table=None, or_table=None, xor_table=None):
    x = np.ascontiguousarray(np.asarray(x_bd, dtype=np.float32)).reshape(TOK, D)
    shards = [
        np.ascontiguousarray(x[c * TOK_PER_CORE : (c + 1) * TOK_PER_CORE])
        for c in range(N_CORES)
    ]
    nc = _get_compiled()
    res = run_on_hw(nc, shards)
    out = np.concatenate([res.results[c]["y"] for c in range(N_CORES)], axis=0)
    return out.reshape(B, S, D).astype(np.float32)


# revision 5
# speedup vs baseline: 1.1162x; 1.1162x over previous
"""Trainium2 Bass kernel for nn_ByteBitwiseFFN.

Reference semantics (per token, D=128 features):
  a = argmax(x[4:20]) + 16*argmax(x[20:36])
  b = argmax(x[36:52]) + 16*argmax(x[52:68])
  res = AND/OR/XOR LUT[a,b] picked by flags x[1]>0.5 / x[2]>0.5 / x[3]>0.5
        (priority AND, OR, XOR; XOR value also used when no flag set)
  active = (x[0]>=0.5) & any-flag; w = active ? 2 : 0
  out = x; out[68 + (res&15)] += w; out[84 + (res>>4)] += w

Key identities:
* Bitwise ops factor over nibbles, so the 256x256 LUTs are never needed:
  res&15 = op(a_lo, b_lo), res>>4 = op(a_hi, b_hi), and for 4-bit operands
  op(u, v) = alpha*(u+v) + beta*(u AND v) with (alpha, beta) =
  (0,1) AND / (1,-1) OR / (1,-2) XOR.  The AND is one int16 bitwise_and.
* First-occurrence argmax without any compare instruction:
  min over j of ((max - x_j) + (j-16)*2^-100) is (argmax-16)*2^-100 --
  the subtract term is 0 exactly at max positions and otherwise dwarfs
  the index encoding (data value gaps are > 1e-6 >> 16*2^-100).

Sharding: pure data parallel over tokens; each of the 8 cores gets
131072/8 = 16384 tokens as its own ExternalInput.

Engine split and layout (from per-op HW microbenchmarks):
* DVE dislikes instructions where both streams are strided/broadcast
  bf16 (falls off the fast path ~15x) -> all small per-token algebra is
  kept in contiguous [P, GT] "plane" tiles, and the per-field argmax
  results are written plane-major by the reduction itself.
* GpSimd is fast on big f32-input streaming ops (and oddly slow on
  small/compare ops) -> it gets the (max - x) subtract pass and the two
  strided f32 one-hot accumulates.
* ACT (scalar engine) does all affine rescales and the flag
  de-interleave via ACTIVATE Copy.
"""

import sys

if "/opt/trn_rl_repo" not in sys.path:
    sys.path.insert(0, "/opt/trn_rl_repo")

import numpy as np

B, S, D = 16, 8192, 128
N_CORES = 8
TOK = B * S                      # 131072 tokens
TOK_PER_CORE = TOK // N_CORES    # 16384
P = 128                          # SBUF partitions

OUT_LO, OUT_HI = 68, 84
EPS = 2.0 ** -100                # index encoding scale for the argmax trick


def build_program(tok_per_core=TOK_PER_CORE, t_per_chunk=32, group=2):
    """Build + compile the single-core SPMD Bass program.

    The core's [tok_per_core, 128] slab is processed in chunks of 128*T
    tokens (contiguous DRAM block <-> SBUF tile [128, T*128]).  Heavy
    streaming passes run per chunk; small per-token algebra runs once per
    group of `group` chunks on concatenated plane tiles.
    """
    import concourse.bass as bass  # noqa: F401
    from concourse import bacc, mybir, tile

    f32 = mybir.dt.float32
    bf16 = mybir.dt.bfloat16
    i16 = mybir.dt.int16
    i32 = mybir.dt.int32
    Op = mybir.AluOpType
    AF = mybir.ActivationFunctionType
    X = mybir.AxisListType.X

    T = t_per_chunk
    chunk_tok = P * T
    assert tok_per_core % (chunk_tok * group) == 0
    n_groups = tok_per_core // (chunk_tok * group)
    GT = group * T                     # tokens-per-partition in one group

    nc = bacc.Bacc(
        "TRN2",
        target_bir_lowering=False,
        debug=False,
        enable_asserts=True,
        num_devices=N_CORES,
    )
    x_dram = nc.dram_tensor("x", [tok_per_core, D], f32, kind="ExternalInput").ap()
    y_dram = nc.dram_tensor("y", [tok_per_core, D], f32, kind="ExternalOutput").ap()

    with tile.TileContext(nc) as tc:
        with (
            tc.tile_pool(name="consts", bufs=1) as cpool,
            tc.tile_pool(name="xtiles", bufs=3) as xpool,
            tc.tile_pool(name="big", bufs=2) as bp,
            tc.tile_pool(name="small", bufs=2) as sp,
        ):
            v = nc.vector
            g = nc.gpsimd
            a = nc.scalar

            # --- constants -------------------------------------------------
            # idxe_full[p, t*64 + g*16 + n] = (n - 16) * 2^-100 for the cand
            # pass (contiguous, so the big DVE add keeps its fast mode), and
            # idx16 for the one-hot compares.
            idx_i = cpool.tile([P, 16], i32)
            nc.gpsimd.iota(idx_i[:], [[1, 16]], base=0, channel_multiplier=0)
            idx16 = cpool.tile([P, 16], bf16)
            v.tensor_copy(idx16[:], idx_i[:])
            idxe = cpool.tile([P, 16], bf16)
            v.tensor_scalar(idxe[:], idx16[:], -16.0, EPS, Op.add, Op.mult)
            idxe_full = cpool.tile([P, T * 64], bf16)
            v.tensor_copy(
                idxe_full.rearrange("p (a n) -> p a n", n=16),
                idxe.unsqueeze(1).broadcast_to([P, T * 4, 16]),
            )
            idx16_gb = idx16.unsqueeze(1).broadcast_to([P, GT, 16])

            for gi in range(n_groups):
                xts = []
                # group-wide plane tiles: [P, 4, GT] (field-major), so all
                # downstream algebra reads contiguous [P, GT] planes.
                am_all = sp.tile([P, 4 * GT], bf16, name="am_all")
                am_pl = am_all.rearrange("p (g t) -> p g t", g=4)
                fl_all = sp.tile([P, GT * 4], bf16, name="fl_all")
                fl_g = fl_all.rearrange("p (t f) -> p t f", f=4)

                for ci in range(group):
                    i = gi * group + ci
                    xt = xpool.tile([P, T * D], f32, name="xt")
                    xts.append(xt)
                    src = x_dram[i * chunk_tok : (i + 1) * chunk_tok, :].rearrange(
                        "(p t) f -> p (t f)", p=P
                    )
                    nc.sync.dma_start(xt[:], src)

                    x3 = xt.rearrange("p (t f) -> p t f", f=D)
                    nib = x3[:, :, 4:68].rearrange("p t (g n) -> p t g n", n=16)

                    # --- first-occurrence argmax of each 16-wide field -----
                    rmax = bp.tile([P, T * 4], f32, name="rmax")
                    rmax3 = rmax.rearrange("p (t g) -> p t g", g=4)
                    v.tensor_reduce(rmax3, nib, axis=X, op=Op.max)

                    dsub = bp.tile([P, T * 64], bf16, name="dsub")
                    dsub4 = dsub.rearrange("p (t g n) -> p t g n", g=4, n=16)
                    g.tensor_tensor(
                        dsub4,
                        rmax3.unsqueeze(3).broadcast_to([P, T, 4, 16]),
                        nib,
                        Op.subtract,
                    )

                    cand = bp.tile([P, T * 64], bf16, name="cand")
                    v.tensor_tensor(cand[:], dsub[:], idxe_full[:], Op.add)

                    # am = (argmax-16)*2^-100, written plane-major:
                    # out element (t, g) -> am_pl[:, g, ci*T + t]
                    am_out = am_pl[:, :, ci * T : (ci + 1) * T].transpose([0, 2, 1])
                    v.tensor_reduce(
                        am_out,
                        cand.rearrange("p (t g n) -> p t g n", g=4, n=16),
                        axis=X,
                        op=Op.min,
                    )
                    # flags (>=0.5) for cols 0..3 (interleaved; ACT splits)
                    v.tensor_scalar(
                        fl_g[:, ci * T : (ci + 1) * T, :],
                        x3[:, :, 0:4],
                        0.5,
                        None,
                        Op.is_ge,
                    )

                # --- batched per-token algebra on [P, GT] planes -----------
                nv = sp.tile([P, 4 * GT], bf16, name="nv")
                nv_pl = nv.rearrange("p (g t) -> p g t", g=4)
                a.activation(nv[:], am_all[:], AF.Copy, bias=16.0, scale=2.0 ** 100)
                nvi = sp.tile([P, 4 * GT], i16, name="nvi")
                nvi_pl = nvi.rearrange("p (g t) -> p g t", g=4)
                v.tensor_copy(nvi[:], nv[:])

                # flag planes via ACT de-interleave
                def flag_plane(nm, k):
                    t_ = sp.tile([P, GT], bf16, name=nm)
                    a.activation(t_[:], fl_g[:, :, k : k + 1], AF.Copy,
                                 bias=0.0, scale=1.0)
                    return t_

                mk = flag_plane("mk", 0)
                ia = flag_plane("ia", 1)
                io = flag_plane("io", 2)
                ix = flag_plane("ix", 3)

                alpha = sp.tile([P, GT], bf16, name="alpha")   # 1 - is_and
                a.activation(alpha[:], ia[:], AF.Copy, bias=1.0, scale=-1.0)
                s1 = sp.tile([P, GT], bf16, name="s1")         # 3 - is_or
                a.activation(s1[:], io[:], AF.Copy, bias=3.0, scale=-1.0)
                s3 = sp.tile([P, GT], bf16, name="s3")         # is_or - 2
                a.activation(s3[:], io[:], AF.Copy, bias=-2.0, scale=1.0)
                s2 = sp.tile([P, GT], bf16, name="s2")
                v.tensor_tensor(s2[:], ia[:], s1[:], Op.mult)
                beta = sp.tile([P, GT], bf16, name="beta")     # 1 / -1 / -2
                v.tensor_tensor(beta[:], s2[:], s3[:], Op.add)
                f1 = sp.tile([P, GT], bf16, name="f1")
                v.tensor_tensor(f1[:], ia[:], io[:], Op.add)
                f2 = sp.tile([P, GT], bf16, name="f2")
                v.tensor_tensor(f2[:], f1[:], ix[:], Op.add)
                f3 = sp.tile([P, GT], bf16, name="f3")         # any-flag
                v.tensor_scalar(f3[:], f2[:], 1.0, None, Op.min)
                act = sp.tile([P, GT], bf16, name="act")
                v.tensor_tensor(act[:], mk[:], f3[:], Op.mult)

                # per half (lo: planes 0&2, hi: planes 1&3)
                resg = sp.tile([P, 2 * GT], bf16, name="resg")
                resg_pl = resg.rearrange("p (h t) -> p h t", h=2)
                for h in range(2):
                    qi = sp.tile([P, GT], i16, name=f"qi{h}")
                    v.tensor_tensor(
                        qi[:], nvi_pl[:, h, :], nvi_pl[:, h + 2, :], Op.bitwise_and
                    )
                    qq = sp.tile([P, GT], bf16, name=f"qq{h}")
                    v.tensor_copy(qq[:], qi[:])
                    ss = sp.tile([P, GT], bf16, name=f"ss{h}")
                    v.tensor_tensor(ss[:], nv_pl[:, h, :], nv_pl[:, h + 2, :], Op.add)
                    c1 = sp.tile([P, GT], bf16, name=f"c1{h}")
                    v.tensor_tensor(c1[:], ss[:], alpha[:], Op.mult)
                    c2 = sp.tile([P, GT], bf16, name=f"c2{h}")
                    v.tensor_tensor(c2[:], qq[:], beta[:], Op.mult)
                    c3 = sp.tile([P, GT], bf16, name=f"c3{h}")
                    v.scalar_tensor_tensor(c3[:], act[:], -16.0, c1[:], Op.mult, Op.add)
                    # resg = res + 16*(1-active): out of 0..15 when inactive
                    v.scalar_tensor_tensor(
                        resg_pl[:, h, :], c2[:], 16.0, c3[:], Op.add, Op.add
                    )

                # --- one-hot (batched): eq then 2x scale on ACT ------------
                e2s = []
                for h in range(2):
                    eqh = sp.tile([P, GT * 16], bf16, name=f"eqh{h}")
                    eqh3 = eqh.rearrange("p (t n) -> p t n", n=16)
                    v.tensor_tensor(
                        eqh3,
                        idx16_gb,
                        resg_pl[:, h, :].unsqueeze(2).broadcast_to([P, GT, 16]),
                        Op.is_equal,
                    )
                    e2 = sp.tile([P, GT * 16], bf16, name=f"e2{h}")
                    a.activation(e2[:], eqh[:], AF.Copy, bias=0.0, scale=2.0)
                    e2s.append(e2.rearrange("p (t n) -> p t n", n=16))

                # --- accumulate into x and store, per chunk ----------------
                for ci in range(group):
                    i = gi * group + ci
                    x3 = xts[ci].rearrange("p (t f) -> p t f", f=D)
                    for h, off in enumerate((OUT_LO, OUT_HI)):
                        xs = x3[:, :, off : off + 16]
                        g.tensor_tensor(
                            xs, xs, e2s[h][:, ci * T : (ci + 1) * T, :], Op.add
                        )
                    dst = y_dram[i * chunk_tok : (i + 1) * chunk_tok, :].rearrange(
                        "(p t) f -> p (t f)", p=P
                    )
                    nc.sync.dma_start(dst, xts[ci][:])

    nc.compile()
    return nc


_compiled = None


def _get_compiled():
    global _compiled
    if _compiled is None:
        _compiled = build_program()
    return _compiled


def run_on_hw(nc, shards, trace=False, **kw):
    from concourse.bass_utils import run_bass_kernel_spmd

    return run_bass_kernel_spmd(
        nc, [{"x": s} for s in shards], list(range(N_CORES)), trace=trace, **kw
    )


def kernel(x_bd, and_table=None, or_table=None, xor_table=None):
    x = np.ascontiguousarray(np.asarray(x_bd, dtype=np.float32)).reshape(TOK, D)
    shards = [
        np.ascontiguousarray(x[c * TOK_PER_CORE : (c + 1) * TOK_PER_CORE])
        for c in range(N_CORES)
    ]
    nc = _get_compiled()
    res = run_on_hw(nc, shards)
    out = np.concatenate([res.results[c]["y"] for c in range(N_CORES)], axis=0)
    return out.reshape(B, S, D).astype(np.float32)


# revision 6
# speedup vs baseline: 1.3799x; 1.2362x over previous
"""Trainium2 Bass kernel for nn_ByteBitwiseFFN.

Reference semantics (per token, D=128 features):
  a = argmax(x[4:20]) + 16*argmax(x[20:36])
  b = argmax(x[36:52]) + 16*argmax(x[52:68])
  res = AND/OR/XOR LUT[a,b] picked by flags x[1]>0.5 / x[2]>0.5 / x[3]>0.5
        (priority AND, OR, XOR; XOR value also used when no flag set)
  active = (x[0]>=0.5) & any-flag; w = active ? 2 : 0
  out = x; out[68 + (res&15)] += w; out[84 + (res>>4)] += w

Key identities:
* Bitwise ops factor over nibbles, so the 256x256 LUTs are never needed:
  res&15 = op(a_lo, b_lo), res>>4 = op(a_hi, b_hi), and for 4-bit operands
  op(u, v) = alpha*(u+v) + beta*(u AND v) with (alpha, beta) =
  (0,1) AND / (1,-1) OR / (1,-2) XOR.  The AND is one int16 bitwise_and.
* First-occurrence argmax without any compare instruction:
  min over j of ((max - x_j) + (j-16)*2^-100) is (argmax-16)*2^-100 --
  the subtract term is 0 exactly at max positions and otherwise dwarfs
  the index encoding (data value gaps are > 1e-6 >> 16*2^-100).

Sharding: pure data parallel over tokens; each of the 8 cores gets
131072/8 = 16384 tokens as its own ExternalInput.

Engine split and layout (from per-op HW microbenchmarks):
* DVE dislikes instructions where both streams are strided/broadcast
  bf16 (falls off the fast path ~15x) -> all small per-token algebra is
  kept in contiguous [P, GT] "plane" tiles, and the per-field argmax
  results are written plane-major by the reduction itself.
* GpSimd is fast on big f32-input streaming ops (and oddly slow on
  small/compare ops) -> it gets the (max - x) subtract pass and the two
  strided f32 one-hot accumulates.
* ACT (scalar engine) does all affine rescales and the flag
  de-interleave via ACTIVATE Copy.
"""

import sys

if "/opt/trn_rl_repo" not in sys.path:
    sys.path.insert(0, "/opt/trn_rl_repo")

import numpy as np

B, S, D = 16, 8192, 128
N_CORES = 8
TOK = B * S                      # 131072 tokens
TOK_PER_CORE = TOK // N_CORES    # 16384
P = 128                          # SBUF partitions

OUT_LO, OUT_HI = 68, 84
EPS = 2.0 ** -100                # index encoding scale for the argmax trick


def build_program(tok_per_core=TOK_PER_CORE, t_per_chunk=32, group=2):
    """Build + compile the single-core SPMD Bass program.

    The core's [tok_per_core, 128] slab is processed in chunks of 128*T
    tokens (contiguous DRAM block <-> SBUF tile [128, T*128]).  Heavy
    streaming passes run per chunk; small per-token algebra runs once per
    group of `group` chunks on concatenated plane tiles.
    """
    import concourse.bass as bass  # noqa: F401
    from concourse import bacc, mybir, tile

    f32 = mybir.dt.float32
    bf16 = mybir.dt.bfloat16
    i16 = mybir.dt.int16
    i32 = mybir.dt.int32
    Op = mybir.AluOpType
    AF = mybir.ActivationFunctionType
    X = mybir.AxisListType.X

    T = t_per_chunk
    chunk_tok = P * T
    assert tok_per_core % (chunk_tok * group) == 0
    n_groups = tok_per_core // (chunk_tok * group)
    GT = group * T                     # tokens-per-partition in one group

    nc = bacc.Bacc(
        "TRN2",
        target_bir_lowering=False,
        debug=False,
        enable_asserts=True,
        num_devices=N_CORES,
    )
    x_dram = nc.dram_tensor("x", [tok_per_core, D], f32, kind="ExternalInput").ap()
    y_dram = nc.dram_tensor("y", [tok_per_core, D], f32, kind="ExternalOutput").ap()

    with tile.TileContext(nc) as tc:
        with (
            tc.tile_pool(name="consts", bufs=1) as cpool,
            tc.tile_pool(name="xtiles", bufs=4) as xpool,
            tc.tile_pool(name="big", bufs=3) as bp,
            tc.tile_pool(name="small", bufs=2) as sp,
        ):
            v = nc.vector
            g = nc.gpsimd
            a = nc.scalar

            # --- constants -------------------------------------------------
            # idxe_full[p, t*64 + g*16 + n] = (n - 16) * 2^-100 for the cand
            # pass (contiguous, so the big DVE add keeps its fast mode), and
            # idx16 for the one-hot compares.
            idx_i = cpool.tile([P, 16], i32)
            nc.gpsimd.iota(idx_i[:], [[1, 16]], base=0, channel_multiplier=0)
            idx16 = cpool.tile([P, 16], bf16)
            v.tensor_copy(idx16[:], idx_i[:])
            idxe = cpool.tile([P, 16], bf16)
            v.tensor_scalar(idxe[:], idx16[:], -16.0, EPS, Op.add, Op.mult)
            idxe_full = cpool.tile([P, T * 64], bf16)
            v.tensor_copy(
                idxe_full.rearrange("p (a n) -> p a n", n=16),
                idxe.unsqueeze(1).broadcast_to([P, T * 4, 16]),
            )
            idx16_gb = idx16.unsqueeze(1).broadcast_to([P, GT, 16])

            for gi in range(n_groups):
                xts = []
                # group-wide plane tiles: [P, 4, GT] (field-major), so all
                # downstream algebra reads contiguous [P, GT] planes.
                am_all = sp.tile([P, 4 * GT], bf16, name="am_all")
                am_pl = am_all.rearrange("p (g t) -> p g t", g=4)
                fl_all = sp.tile([P, GT * 4], bf16, name="fl_all")
                fl_g = fl_all.rearrange("p (t f) -> p t f", f=4)

                for ci in range(group):
                    i = gi * group + ci
                    xt = xpool.tile([P, T * D], f32, name="xt")
                    xts.append(xt)
                    src = x_dram[i * chunk_tok : (i + 1) * chunk_tok, :].rearrange(
                        "(p t) f -> p (t f)", p=P
                    )
                    nc.sync.dma_start(xt[:], src)

                    x3 = xt.rearrange("p (t f) -> p t f", f=D)
                    nib = x3[:, :, 4:68].rearrange("p t (g n) -> p t g n", n=16)

                    # --- first-occurrence argmax of each 16-wide field -----
                    rmax = bp.tile([P, T * 4], f32, name="rmax")
                    rmax3 = rmax.rearrange("p (t g) -> p t g", g=4)
                    v.tensor_reduce(rmax3, nib, axis=X, op=Op.max)

                    dsub = bp.tile([P, T * 64], bf16, name="dsub")
                    dsub4 = dsub.rearrange("p (t g n) -> p t g n", g=4, n=16)
                    g.tensor_tensor(
                        dsub4,
                        rmax3.unsqueeze(3).broadcast_to([P, T, 4, 16]),
                        nib,
                        Op.subtract,
                    )

                    cand = bp.tile([P, T * 64], bf16, name="cand")
                    v.tensor_tensor(cand[:], dsub[:], idxe_full[:], Op.add)

                    # am = (argmax-16)*2^-100, written plane-major:
                    # out element (t, g) -> am_pl[:, g, ci*T + t]
                    am_out = am_pl[:, :, ci * T : (ci + 1) * T].transpose([0, 2, 1])
                    v.tensor_reduce(
                        am_out,
                        cand.rearrange("p (t g n) -> p t g n", g=4, n=16),
                        axis=X,
                        op=Op.min,
                    )
                    # flags (>=0.5) for cols 0..3 (interleaved; ACT splits)
                    v.tensor_scalar(
                        fl_g[:, ci * T : (ci + 1) * T, :],
                        x3[:, :, 0:4],
                        0.5,
                        None,
                        Op.is_ge,
                    )

                # --- batched per-token algebra on [P, GT] planes -----------
                nv = sp.tile([P, 4 * GT], bf16, name="nv")
                nv_pl = nv.rearrange("p (g t) -> p g t", g=4)
                a.activation(nv[:], am_all[:], AF.Copy, bias=16.0, scale=2.0 ** 100)
                nvi = sp.tile([P, 4 * GT], i16, name="nvi")
                nvi_pl = nvi.rearrange("p (g t) -> p g t", g=4)
                v.tensor_copy(nvi[:], nv[:])

                # flag planes via ACT de-interleave
                def flag_plane(nm, k):
                    t_ = sp.tile([P, GT], bf16, name=nm)
                    a.activation(t_[:], fl_g[:, :, k : k + 1], AF.Copy,
                                 bias=0.0, scale=1.0)
                    return t_

                mk = flag_plane("mk", 0)
                ia = flag_plane("ia", 1)
                io = flag_plane("io", 2)
                ix = flag_plane("ix", 3)

                alpha = sp.tile([P, GT], bf16, name="alpha")   # 1 - is_and
                a.activation(alpha[:], ia[:], AF.Copy, bias=1.0, scale=-1.0)
                s1 = sp.tile([P, GT], bf16, name="s1")         # 3 - is_or
                a.activation(s1[:], io[:], AF.Copy, bias=3.0, scale=-1.0)
                s3 = sp.tile([P, GT], bf16, name="s3")         # is_or - 2
                a.activation(s3[:], io[:], AF.Copy, bias=-2.0, scale=1.0)
                s2 = sp.tile([P, GT], bf16, name="s2")
                v.tensor_tensor(s2[:], ia[:], s1[:], Op.mult)
                beta = sp.tile([P, GT], bf16, name="beta")     # 1 / -1 / -2
                v.tensor_tensor(beta[:], s2[:], s3[:], Op.add)
                f1 = sp.tile([P, GT], bf16, name="f1")
                v.tensor_tensor(f1[:], ia[:], io[:], Op.add)
                f2 = sp.tile([P, GT], bf16, name="f2")
                v.tensor_tensor(f2[:], f1[:], ix[:], Op.add)
                f3 = sp.tile([P, GT], bf16, name="f3")         # any-flag
                v.tensor_scalar(f3[:], f2[:], 1.0, None, Op.min)
                act = sp.tile([P, GT], bf16, name="act")
                v.tensor_tensor(act[:], mk[:], f3[:], Op.mult)

                # per half (lo: planes 0&2, hi: planes 1&3)
                resg = sp.tile([P, 2 * GT], bf16, name="resg")
                resg_pl = resg.rearrange("p (h t) -> p h t", h=2)
                for h in range(2):
                    qi = sp.tile([P, GT], i16, name=f"qi{h}")
                    v.tensor_tensor(
                        qi[:], nvi_pl[:, h, :], nvi_pl[:, h + 2, :], Op.bitwise_and
                    )
                    qq = sp.tile([P, GT], bf16, name=f"qq{h}")
                    v.tensor_copy(qq[:], qi[:])
                    ss = sp.tile([P, GT], bf16, name=f"ss{h}")
                    v.tensor_tensor(ss[:], nv_pl[:, h, :], nv_pl[:, h + 2, :], Op.add)
                    c1 = sp.tile([P, GT], bf16, name=f"c1{h}")
                    v.tensor_tensor(c1[:], ss[:], alpha[:], Op.mult)
                    c2 = sp.tile([P, GT], bf16, name=f"c2{h}")
                    v.tensor_tensor(c2[:], qq[:], beta[:], Op.mult)
                    c3 = sp.tile([P, GT], bf16, name=f"c3{h}")
                    v.scalar_tensor_tensor(c3[:], act[:], -16.0, c1[:], Op.mult, Op.add)
                    # resg = res + 16*(1-active): out of 0..15 when inactive
                    v.scalar_tensor_tensor(
                        resg_pl[:, h, :], c2[:], 16.0, c3[:], Op.add, Op.add
                    )

                # --- one-hot (batched): eq then 2x scale on ACT ------------
                e2s = []
                for h in range(2):
                    eqh = sp.tile([P, GT * 16], bf16, name=f"eqh{h}")
                    eqh3 = eqh.rearrange("p (t n) -> p t n", n=16)
                    v.tensor_tensor(
                        eqh3,
                        idx16_gb,
                        resg_pl[:, h, :].unsqueeze(2).broadcast_to([P, GT, 16]),
                        Op.is_equal,
                    )
                    e2 = sp.tile([P, GT * 16], bf16, name=f"e2{h}")
                    a.activation(e2[:], eqh[:], AF.Copy, bias=0.0, scale=2.0)
                    e2s.append(e2.rearrange("p (t n) -> p t n", n=16))

                # --- accumulate into x and store, per chunk ----------------
                for ci in range(group):
                    i = gi * group + ci
                    x3 = xts[ci].rearrange("p (t f) -> p t f", f=D)
                    for h, off in enumerate((OUT_LO, OUT_HI)):
                        xs = x3[:, :, off : off + 16]
                        g.tensor_tensor(
                            xs, xs, e2s[h][:, ci * T : (ci + 1) * T, :], Op.add
                        )
                    dst = y_dram[i * chunk_tok : (i + 1) * chunk_tok, :].rearrange(
                        "(p t) f -> p (t f)", p=P
                    )
                    nc.sync.dma_start(dst, xts[ci][:])

    nc.compile()
    return nc


_compiled = None


def _get_compiled():
    global _compiled
    if _compiled is None:
        _compiled = build_program()
    return _compiled


def run_on_hw(nc, shards, trace=False, **kw):
    from concourse.bass_utils import run_bass_kernel_spmd

    return run_bass_kernel_spmd(
        nc, [{"x": s} for s in shards], list(range(N_CORES)), trace=trace, **kw
    )


def kernel(x_bd, and_table=None, or_table=None, xor_table=None):
    x = np.ascontiguousarray(np.asarray(x_bd, dtype=np.float32)).reshape(TOK, D)
    shards = [
        np.ascontiguousarray(x[c * TOK_PER_CORE : (c + 1) * TOK_PER_CORE])
        for c in range(N_CORES)
    ]
    nc = _get_compiled()
    res = run_on_hw(nc, shards)
    out = np.concatenate([res.results[c]["y"] for c in range(N_CORES)], axis=0)
    return out.reshape(B, S, D).astype(np.float32)
